# revision 42
# baseline (speedup 1.0000x reference)
"""Trainium2 Bass kernel for nn_DecoderLayer_11424613007924.

DecoderLayer: block-sparse attention (BLEN=256, causal first block,
2D-local windowed tail blocks) + LayerNorm + MLP, fp32 I/O.

Sharding: data-parallel over batch. 32 samples -> 8 NeuronCores x 4.

Per-core dataflow (per sample):
  X [1024,256] --(cast bf16, DMA-transpose)--> X.T
  Q.T/K.T = W.T @ X.T (PE, bf16);  V = X.T-stationary @ Wv (token-major)
  per (block b, query-chunk qc):
      for each 128-aligned key chunk of the window:
        S.T = kT-chunk.T @ qT-chunk  (PE, 4 heads packed via tile_position
              row strips; out [keys<=128, 8h x 128q] PSUM)
        P.T = exp(S.T) (ACT, PSUM->SBUF bf16, head-strided)
        P.T *= 0/1 mask.T (DVE, in place)   <- no DMA transpose of P needed
      O_u = P.T-chunks.T @ v (PE, ones-col in V emits denominators)
      O = O_u * recip(den) (DVE)
  y = O.T-stationary @ Wo; z1 = y + X (fp32)
  LN1 via bn_stats; rstd = exp(-0.5*ln(var+eps)) on ACT (stays in the
  exp/ln/relu act table -> zero ACT_TABLE_LOADs); z1n = (z1-m)*rstd
  h = z1n.T @ W1eff + b1eff (ones-row inject); r = relu(h)
  z2 = r.T @ W2 + (ln1_b+b2) inject + z1n*g1 inject (diag(g1) matmul)
  out = (z2-m2)*g2b*rstd2 + b2gb  (scalar_tensor_tensor x2)

Numerics: matmul operands bf16 (PSUM fp32 accum); residual stream,
LN stats, softmax denominators fp32. Masks are multiplicative 0/1 on
exp(S) (masked entries exp(S)*0 = 0).
"""
import numpy as np
import ml_dtypes

import concourse.bass as bass
import concourse.mybir as mybir
import concourse.tile as tile
from bass_rust import ScopedClock

BF = mybir.dt.bfloat16
F32 = mybir.dt.float32
AF = mybir.ActivationFunctionType
OP = mybir.AluOpType

N_CORES = 8
B, L, D = 32, 1024, 256
H, DH, F = 8, 32, 512
BLEN = 256
SPC = B // N_CORES  # samples per core
NB = L // BLEN      # 4 blocks
EPS = 1e-6

# ---------------------------------------------------------------- fixups ---
# This container's walrus build rejects instructions carrying >1 semaphore
# wait. Split extra waits onto same-engine NOPs after Tile scheduling.


def _split_sync_waits(nc):
    for fn in nc.m.functions:
        for bb in fn.blocks:
            insts = bb.instructions
            if not insts:
                continue
            new_list = []
            n_split = 0
            for inst in insts:
                si = inst.sync_info
                waits = list(si.on_wait) if (si and si.on_wait) else []
                if len(waits) > 1:
                    si.on_wait = waits[:1]
                    for w in waits[1:]:
                        nop = nc.engines[inst.engine].nop()
                        for f2 in nc.m.functions:
                            for b2 in f2.blocks:
                                l2 = b2.instructions
                                if l2 and l2[-1] is nop.ins:
                                    l2.pop()
                        nop.ins.sync_info = mybir.SyncInfo(on_wait=[w], on_update=[])
                        new_list.append(nop.ins)
                        n_split += 1
                new_list.append(inst)
            if n_split:
                bb.instructions = new_list


def _patched_drain_and_barrier(self, tick_clock, wait_clock):
    nc = self.nc
    probe = nc.sync.nop()
    wait_clock.add_sem_waits(probe.ins, ScopedClock({None: tick_clock.global_clock}))
    nc.sync.drain()
    nc.all_engine_barrier()
    assert self.sems is not None
    popped = nc._tile_sem_poison_stack.pop()
    assert popped is self._sem_poison
    nc.clear_and_free_semaphores(list(self.sems.allocated().values()))
    nc.all_engine_barrier()


tile.TileContext._drain_and_barrier = _patched_drain_and_barrier


# ------------------------------------------------------------- host prep ---

def _mask2d(blen=BLEN, h=32, win=6):
    cp = np.arange(blen, 2 * blen)[:, None]
    op = np.arange(2 * blen)[None, :]
    causal = op <= cp
    ch, cw = cp // h, cp % h
    oh, ow = op // h, op % h
    ok = causal & (np.abs(ch - oh) <= win) & (np.abs(cw - ow) <= win)
    return np.where(ok, 0.0, -30.0).astype(np.float32)


def _bcast(ap, offset_extra, plist):
    """AP with explicit [step,count] free dims appended to partition dim."""
    return bass.AP(tensor=ap.tensor, offset=ap.offset + offset_extra,
                   ap=[list(ap.ap[0])] + plist)


def _mkap(ap, offset_extra, dims):
    """AP with fully explicit [step,count] dims (incl. partition dim)."""
    return bass.AP(tensor=ap.tensor, offset=ap.offset + offset_extra, ap=dims)


def _chunks_of(b, qc):
    """128-aligned key chunk starts of the (extended) window for (b, qc).

    Tail windows are widened 64 keys left so every chunk is a full
    128-aligned 128-row chunk; the extra keys are provably masked (they
    violate the |ch-oh|<=6 row constraint) and the mask zeroes them.
    """
    if b == 0:
        return [0] if qc == 0 else [0, 128]
    m0 = 2 * (b - 1) + qc
    return [128 * m0, 128 * (m0 + 1), 128 * (m0 + 2)]


_CACHE = {}

# tuning knobs (read at build time)
# psS is one 4-bank tile; psO tolerates a single buffer (Ou(N) alloc comes
# ~2.5us after norm(N-1) is issued); the third psM bank unblocks the
# MLP-down/QKV matmul chains. 4 + 1 + 3 = 8 banks.
KNOBS = {"psS": 1, "psO": 1, "psM": 3, "big": 2, "small": 2}


def _build():
    if "nc" in _CACHE:
        return _CACHE["nc"]
    nc = bass.Bass(target_bir_lowering=False)

    xin = nc.declare_dram_parameter("X", [SPC, L, D], F32, isOutput=False)
    out = nc.declare_dram_parameter("OUT", [SPC, L, D], F32, isOutput=True)
    wq = nc.declare_dram_parameter("wq", [D, D], BF, isOutput=False)
    wk = nc.declare_dram_parameter("wk", [D, D], BF, isOutput=False)
    wv = nc.declare_dram_parameter("wv", [D, D], BF, isOutput=False)
    wo = nc.declare_dram_parameter("wo", [D, D], BF, isOutput=False)
    w1 = nc.declare_dram_parameter("w1", [D, F], BF, isOutput=False)
    w2 = nc.declare_dram_parameter("w2", [F, D], BF, isOutput=False)
    dg1 = nc.declare_dram_parameter("dg1", [2, 128, D], BF, isOutput=False)
    b1col = nc.declare_dram_parameter("b1col", [128, 4], F32, isOutput=False)
    browz = nc.declare_dram_parameter("browz", [1, D], BF, isOutput=False)
    g2b = nc.declare_dram_parameter("g2b", [128, D], F32, isOutput=False)
    b2gb = nc.declare_dram_parameter("b2gb", [128, D], F32, isOutput=False)
    ones1 = nc.declare_dram_parameter("ones1", [1, 128], BF, isOutput=False)
    mtail = nc.declare_dram_parameter("mtail", [2, 128, 384], BF, isOutput=False)
    mtb0 = nc.declare_dram_parameter("mtb0", [128, 128], BF, isOutput=False)

    with tile.TileContext(nc) as tc:
        with (
            tc.tile_pool(name="static", bufs=1) as st,
            tc.tile_pool(name="big", bufs=KNOBS["big"]) as bigp,
            tc.tile_pool(name="small", bufs=KNOBS["small"]) as smp,
            tc.tile_pool(name="psS", bufs=KNOBS["psS"], space="PSUM") as psS,
            tc.tile_pool(name="psO", bufs=KNOBS["psO"], space="PSUM") as psO,
            tc.tile_pool(name="psM", bufs=KNOBS["psM"], space="PSUM") as psM,
        ):
            # ---- statics
            # weights stored K-chunked: chunk kc lives at cols [kc*N, (kc+1)*N)
            s_wq = st.tile([128, 2 * D], BF, name="wq", tag="wq")
            s_wk = st.tile([128, 2 * D], BF, name="wk", tag="wk")
            s_wv = st.tile([128, 2 * D], BF, name="wv", tag="wv")
            s_wo = st.tile([128, 2 * D], BF, name="wo", tag="wo")
            s_w1 = st.tile([128, 2 * F], BF, name="w1", tag="w1")
            s_w2 = st.tile([128, 4 * D], BF, name="w2", tag="w2")
            s_dg1 = [st.tile([128, D], BF, name=f"dg1{c}", tag=f"dg1{c}") for c in range(2)]
            s_b1col = st.tile([128, 4], F32, name="b1col", tag="b1col")
            s_browz = st.tile([1, D], BF, name="browz", tag="browz")
            s_g2b = st.tile([128, D], F32, name="g2b", tag="g2b")
            s_b2gb = st.tile([128, D], F32, name="b2gb", tag="b2gb")
            s_ones1 = st.tile([1, 128], BF, name="ones1", tag="ones1")
            s_mT = [st.tile([128, 384], BF, name=f"mT{qc}", tag=f"mT{qc}") for qc in range(2)]
            s_mTb0 = st.tile([128, 128], BF, name="mTb0", tag="mTb0")
            s_eps = st.tile([128, 1], F32, name="eps", tag="eps")
            # qk-projection weights first: sample 0's first matmuls need them
            # statics on the scalar queue: sync stays free for sample 0's
            # X transpose, so compute can start while statics stream in
            for dst, dsrc, nch_, w_ in [
                (s_wq, wq, 2, D), (s_wk, wk, 2, D), (s_wv, wv, 2, D),
                (s_wo, wo, 2, D), (s_w1, w1, 2, F), (s_w2, w2, 4, D),
            ]:
                for kc in range(nch_):
                    nc.scalar.dma_start(out=dst[:, kc * w_:(kc + 1) * w_],
                                        in_=dsrc[kc * 128:(kc + 1) * 128, :])
            for dst, dsrc in [
                (s_mTb0, mtb0), (s_ones1, ones1),
                (s_b1col, b1col), (s_browz, browz),
                (s_g2b, g2b), (s_b2gb, b2gb),
            ]:
                nc.scalar.dma_start(out=dst[:], in_=dsrc[:])
            nc.vector.memset(s_eps[:], EPS)
            for c in range(2):
                nc.scalar.dma_start(out=s_dg1[c][:], in_=dg1[c])
                nc.scalar.dma_start(out=s_mT[c][:], in_=mtail[c])

            # weight chunk kc (K-rows kc*128..) cols [mlo,mhi) of a w_-wide chunk
            def wch(t, kc, w_, mlo, mhi):
                return t[:, kc * w_ + mlo:kc * w_ + mhi]

            # ---- X load / cast / transpose, software-pipelined -------------
            # load runs 2 samples ahead (gpsimd queue), cast 1 ahead, and the
            # transpose for s+1 is issued on sync just before s's store.
            xt_next, xb_next, xT_next = {}, {}, {}

            def load_x(s):
                xt_all = bigp.tile([128, 8 * D], F32, name="xt_all",
                                   tag="xt_all", bufs=3)
                # scalar-queue HWDGE for steady-state loads: the gpsimd
                # SWDGE path drip-feeds 1024 descriptors for ~100us and
                # starves the sync-queue transposes of DMA engines. The
                # first two loads stay on gpsimd so they don't queue
                # behind the statics on the scalar queue.
                eng = nc.gpsimd if s < 2 else nc.scalar
                eng.dma_start(
                    out=xt_all[:],
                    in_=_mkap(xin[s, 0:1, 0:1], 0,
                              [[D, 128], [128 * D, 8], [1, D]]))
                xt_next[s] = xt_all

            def cast_x(s):
                # xb_all col = dc*1024 + tc*128 + p' (C-major, C = dc*8+tc)
                # so ONE batched transpose yields xTall col = C*128 + j
                #   = dc*1024 + tc*128 + j  (the layout consumers expect)
                # startup samples cast on DVE (idle then, 6x faster than the
                # gpsimd software cast); steady-state on the idle gpsimd
                xb_all = bigp.tile([128, 8 * D], BF, name="xb_all", tag="xb_all")
                eng = nc.vector if s < 2 else nc.gpsimd
                eng.tensor_copy(
                    _bcast(xb_all[0:128, 0:1], 0,
                           [[128, 8], [1024, 2], [1, 128]]),
                    _bcast(xt_next[s][0:128, 0:1], 0,
                           [[256, 8], [128, 2], [1, 128]]))
                xb_next[s] = xb_all

            def xpose_x(s):
                xTall = bigp.tile([128, 2 * L], BF, name="xTall", tag="xTall")
                nc.sync.dma_start_transpose(
                    out=_bcast(xTall[0:128, 0:1], 0, [[128, 16], [1, 128]]),
                    in_=xb_next.pop(s)[:])
                xT_next[s] = xTall

            load_x(0)
            if SPC > 1:
                load_x(1)
            cast_x(0)
            xpose_x(0)
            # ---- Q.T / K.T (d-major) and V (token-major), emitted one
            # sample ahead (between MLP-up and MLP-down of the previous
            # sample) so the PE never waits for the LN2 chain to free psM.
            qkv_next = {}

            def qkv_phase(s):
                xTall = xT_next[s]
                qT = [bigp.tile([128, L], BF, name=f"qT{mc}", tag=f"qT{mc}") for mc in range(2)]
                kT = [bigp.tile([128, L], BF, name=f"kT{mc}", tag=f"kT{mc}") for mc in range(2)]
                for wt, dstl in ((s_wq, qT), (s_wk, kT)):
                    for mc in range(2):
                        for hf in range(2):
                            ps = psM.tile([128, 512], F32, name="mm", tag="mm")
                            for kc in range(2):
                                nc.tensor.matmul(
                                    ps[:], lhsT=wch(wt, kc, D, mc * 128, mc * 128 + 128),
                                    rhs=xTall[:, kc * L + hf * 512:kc * L + (hf + 1) * 512],
                                    start=(kc == 0), stop=(kc == 1))
                            nc.scalar.copy(
                                dstl[mc][:, hf * 512:(hf + 1) * 512], ps[:])
                # head h%4==3 sits at base partition 96 (invalid for matmul
                # operands); extract to offset-0 tiles via SBUF->SBUF DMA
                q3 = [smp.tile([32, L], BF, name=f"q3_{hc}", tag=f"q3_{hc}") for hc in range(2)]
                k3 = [smp.tile([32, L], BF, name=f"k3_{hc}", tag=f"k3_{hc}") for hc in range(2)]
                for hc in range(2):
                    nc.sync.dma_start(out=q3[hc][:], in_=qT[hc][96:128, :])
                    nc.sync.dma_start(out=k3[hc][:], in_=kT[hc][96:128, :])
                # vb: 0-aligned V token-chunks, layout [128, 8*33]: head h at
                # cols h*33..h*33+32, col h*33+32 is ones -> AV matmul emits
                # softmax denominators for free.
                vb = [bigp.tile([128, 264], BF, name=f"vb{t}", tag=f"vb{t}") for t in range(8)]
                for t in range(8):
                    ps = psM.tile([128, D], F32, name="mm", tag="mm")
                    for kc in range(2):
                        nc.tensor.matmul(
                            ps[:], lhsT=xTall[:, kc * L + t * 128:kc * L + (t + 1) * 128],
                            rhs=s_wv[:, kc * D:(kc + 1) * D],
                            start=(kc == 0), stop=(kc == 1))
                    # on ACT: DVE is the busier engine in this window
                    nc.scalar.copy(
                        _bcast(vb[t][0:128, 0:1], 0, [[33, 8], [1, 32]]),
                        ps[:])
                    nc.gpsimd.memset(
                        _bcast(vb[t][0:128, 0:1], 32, [[33, 8]]), 1.0)
                qkv_next[s] = (qT, kT, q3, k3, vb)

            # rstd everywhere = exp(-0.5*ln(var+eps)): stays inside the
            # exp/ln ACT table -> no ACT_TABLE_LOAD churn (sqrt lives in
            # another table and would force a 1.3us reload around every LN).
            qkv_phase(0)
            for s in range(SPC):
                if s + 2 < SPC:
                    load_x(s + 2)
                if s + 1 < SPC:
                    cast_x(s + 1)
                xt_all = xt_next.pop(s)
                xt = [xt_all[:, tc_ * D:(tc_ + 1) * D] for tc_ in range(8)]
                xTall = xT_next.pop(s)
                qT, kT, q3, k3, vb = qkv_next.pop(s)

                # ---- attention --------------------------------------------
                # S computed TRANSPOSED (keys on partitions): the exp output
                # IS the AV lhsT, so no per-block P transposes are needed.
                # AV for block-pair N is emitted after S.T of block-pair N+1,
                # so the PE's in-order queue stays busy while exp/mask of
                # N+1 are in flight on ACT/DVE.
                # O stored per token-half, col = (tc%4)*256 + dc*128 + d%128,
                # so each half can DMA-transpose as soon as its 4 block-pairs
                # finish: half 0 transposes while attention of blocks 2-3 is
                # still running, and Wo tc0-3 starts with zero transpose wait.
                Oh = [bigp.tile([128, L], BF, name=f"Oh{hf}", tag=f"Oh{hf}")
                      for hf in range(2)]
                OTh = [bigp.tile([128, L], BF, name=f"OTh{hf}", tag=f"OTh{hf}")
                       for hf in range(2)]
                av_ou = {}

                def xpose_o(hf):
                    nc.sync.dma_start_transpose(
                        out=_bcast(OTh[hf][0:128, 0:1], 0, [[128, 8], [1, 128]]),
                        in_=Oh[hf][:])

                def av_half(p, half):
                    # AV for heads [half*4, half*4+4); half 1 also emits the
                    # denominator-normalized write into Oall.
                    b, qc, chunks, PT = p
                    nch = len(chunks)
                    if half == 0:
                        av_ou[0] = psO.tile([128, 264], F32, name="Ou", tag="Ou")
                    Ou = av_ou[0]
                    for hi in range(4):
                        h = half * 4 + hi
                        for ci, kst in enumerate(chunks):
                            nc.tensor.matmul(
                                Ou[:, h * 33:(h + 1) * 33],
                                lhsT=PT[:, h * 384 + ci * 128:h * 384 + (ci + 1) * 128],
                                rhs=vb[kst // 128][:, h * 33:(h + 1) * 33],
                                start=(ci == 0), stop=(ci == nch - 1))
                    if half == 1:
                        rec = smp.tile([128, 8], F32, name="rec", tag="rec")
                        nc.vector.reciprocal(
                            rec[:], _bcast(Ou[0:128, 0:1], 32, [[33, 8]]))
                        tc_o = 2 * b + qc
                        nc.vector.tensor_tensor(
                            out=_bcast(Oh[tc_o // 4][0:128, 0:1],
                                       (tc_o % 4) * 256,
                                       [[128, 2], [32, 4], [1, 32]]),
                            in0=_bcast(Ou[0:128, 0:1], 0,
                                       [[132, 2], [33, 4], [1, 32]]),
                            in1=_bcast(rec[0:128, 0:1], 0,
                                       [[4, 2], [1, 4], [0, 32]]),
                            op=OP.mult)

                pend = None
                for b in range(NB):
                    for qc in range(2):
                        chunks = _chunks_of(b, qc)
                        nch = len(chunks)
                        qlo = b * 256 + qc * 128
                        # PT: head h, chunk ci at cols h*384 + ci*128
                        PT = bigp.tile([128, 8 * 384], BF, name="PT", tag="PT")

                        def st_mm(ps, hc, hi, kst, cw):
                            hr = hi * 32
                            if hi == 3:
                                ksl = k3[hc][0:32, kst:kst + 128]
                                qsl = q3[hc][0:32, qlo:qlo + 128]
                                tp = (0, 0)
                            else:
                                ksl = kT[hc][hr:hr + 32, kst:kst + 128]
                                qsl = qT[hc][hr:hr + 32, qlo:qlo + 128]
                                tp = (hr, 0)
                            nc.tensor.matmul(
                                ps[:, cw * 128:(cw + 1) * 128],
                                lhsT=ksl, rhs=qsl,
                                start=True, stop=True, tile_position=tp)

                        cur = (b, qc, chunks, PT)
                        for hg in range(2):
                            # one 4-bank tile: packed head hi owns bank hi
                            # (concurrent row-strip matmuls must not share a
                            # PSUM bank -> hi's chunks at cols hi*512+ci*128)
                            ps = psS.tile([128, 2048], F32, name="S", tag="S")
                            for ci, kst in enumerate(chunks):
                                for hi in range(4):
                                    st_mm(ps, hg, hi, kst, hi * 4 + ci)
                            # single exp per head-group (PSUM -> SBUF bf16)
                            nc.scalar.activation(
                                out=_bcast(PT[0:128, 0:1], hg * 1536,
                                           [[384, 4], [1, nch * 128]]),
                                in_=_bcast(ps[0:128, 0:1], 0,
                                           [[512, 4], [1, nch * 128]]),
                                func=AF.Exp)
                            # 0/1 masks, in place
                            if b == 0:
                                for ci in range(nch):
                                    if qc == 1 and ci == 0:
                                        continue  # all-valid chunk
                                    nc.vector.tensor_tensor(
                                        out=_bcast(PT[0:128, 0:1],
                                                   hg * 1536 + ci * 128,
                                                   [[384, 4], [1, 128]]),
                                        in0=_bcast(PT[0:128, 0:1],
                                                   hg * 1536 + ci * 128,
                                                   [[384, 4], [1, 128]]),
                                        in1=_bcast(s_mTb0[0:128, 0:1], 0,
                                                   [[0, 4], [1, 128]]),
                                        op=OP.mult)
                            else:
                                nc.vector.tensor_tensor(
                                    out=_bcast(PT[0:128, 0:1], hg * 1536,
                                               [[384, 4], [1, 384]]),
                                    in0=_bcast(PT[0:128, 0:1], hg * 1536,
                                               [[384, 4], [1, 384]]),
                                    in1=_bcast(s_mT[qc][0:128, 0:1], 0,
                                               [[0, 4], [1, 384]]),
                                    op=OP.mult)
                            # keep the PE busy while exp/mask drain: finish
                            # the previous block-pair's AV after hg0, start
                            # this one's after hg1
                            if hg == 0:
                                if pend is not None:
                                    av_half(pend, 1)
                                    if pend[0] == 1 and pend[1] == 1:
                                        xpose_o(0)
                            else:
                                av_half(cur, 0)
                        pend = cur
                av_half(pend, 1)
                xpose_o(1)

                # ---- Wo + residual + LN1 ----------------------------------
                # z1n_all col = hf*1024 + dc*512 + (tc%4)*128 + p'; its
                # transpose is emitted per token-half (after tc 3 and 7) so
                # MLP-up's first half starts before LN1 fully drains.
                # z1nTall col = hf*1024 + dc*512 + tc'*128 + j.
                # rstd is batched per half (ONE Ln + ONE Exp on [128,4]
                # instead of 8 tiny ACT ops).
                z1n_all = bigp.tile([128, 8 * D], BF, name="z1n_all", tag="z1n_all")
                # one tile per token-half: MLP-down's hf0 injects must not
                # carry a false tile-level dependency on the hf1 transpose
                z1nT_h = [bigp.tile([128, L], BF, name=f"z1nT{hf}", tag=f"z1nT{hf}")
                          for hf in range(2)]
                # z1n stored tc-major within each half (col = k*256 + dc*128
                # + j) so each half transposes in two quarter-pieces; the
                # first fires after just two token-chunks, and rstd is
                # computed per pair, shortening the LN1 tail before MLP-up.
                for hf in range(2):
                    st2h = smp.tile([128, 8], F32, name="st2h", tag="st2h")
                    rstd4 = smp.tile([128, 4], F32, name="rstd4", tag="rstd4")
                    for p in range(2):
                        z1s = []
                        for k2 in range(2):
                            k = p * 2 + k2
                            tc_ = hf * 4 + k
                            ps = psM.tile([128, D], F32, name="mm", tag="mm")
                            for dc in range(2):
                                cl = (k * 2 + dc) * 128
                                nc.tensor.matmul(
                                    ps[:], lhsT=OTh[tc_ // 4][:, cl:cl + 128],
                                    rhs=s_wo[:, dc * D:(dc + 1) * D],
                                    start=(dc == 0), stop=(dc == 1))
                            z1 = smp.tile([128, D], F32, name="z1", tag="z1",
                                          bufs=4)
                            nc.vector.tensor_tensor(out=z1[:], in0=ps[:],
                                                    in1=xt[tc_][:], op=OP.add)
                            st6 = smp.tile([128, 6], F32, name="st6", tag="st6")
                            nc.vector.bn_stats(st6[:], z1[:])
                            nc.vector.bn_aggr(st2h[:, 2 * k:2 * k + 2], st6[:])
                            z1s.append(z1)
                        lnv2 = smp.tile([128, 2], F32, name="lnv2", tag="lnv2")
                        nc.scalar.activation(
                            out=lnv2[:],
                            in_=_bcast(st2h[0:128, 0:1], 4 * p + 1, [[2, 2]]),
                            func=AF.Ln, bias=s_eps[:, 0:1])
                        nc.scalar.activation(out=rstd4[:, 2 * p:2 * p + 2],
                                             in_=lnv2[:], func=AF.Exp,
                                             scale=-0.5)
                        for k2 in range(2):
                            k = p * 2 + k2
                            nc.vector.tensor_scalar(
                                out=_bcast(z1n_all[0:128, 0:1],
                                           hf * 1024 + k * 256,
                                           [[128, 2], [1, 128]]),
                                in0=z1s[k2][:],
                                scalar1=st2h[:, 2 * k:2 * k + 1],
                                scalar2=rstd4[:, k:k + 1],
                                op0=OP.subtract, op1=OP.mult)
                        nc.sync.dma_start_transpose(
                            out=_bcast(z1nT_h[hf][0:128, 0:1], p * 512,
                                       [[128, 4], [1, 128]]),
                            in_=z1n_all[:, hf * 1024 + p * 512:
                                        hf * 1024 + (p + 1) * 512])

                def z1nT(tc_, dc):
                    c0 = ((tc_ % 4) * 2 + dc) * 128
                    return z1nT_h[tc_ // 4][:, c0:c0 + 128]

                # ---- MLP up (h.T orientation) + fused bias+relu -----------
                # h.T = W1eff.T-chunks @ z1nT; relu(x + b1) with b1 per-
                # partition in this orientation -> no r transpose needed.
                rTall = bigp.tile([128, 4 * L], BF, name="rTall", tag="rTall")
                for hf in range(2):
                    for fc in range(4):
                        ps = psM.tile([128, 512], F32, name="mm", tag="mm")
                        for dc in range(2):
                            nc.tensor.matmul(
                                ps[:],
                                lhsT=s_w1[:, dc * F + fc * 128:dc * F + (fc + 1) * 128],
                                rhs=_bcast(z1nT_h[hf][0:128, 0:1], dc * 128,
                                           [[256, 4], [1, 128]]),
                                start=(dc == 0), stop=(dc == 1))
                        dst = rTall[:, fc * L + hf * 512:fc * L + (hf + 1) * 512]
                        if (fc + hf) % 2 == 0:
                            nc.scalar.activation(out=dst, in_=ps[:], func=AF.Relu,
                                                 bias=s_b1col[:, fc:fc + 1])
                        else:
                            nc.vector.tensor_scalar(
                                out=dst, in0=ps[:], scalar1=s_b1col[:, fc:fc + 1],
                                scalar2=0.0, op0=OP.add, op1=OP.max)

                # ---- MLP down + injects + LN2 + out -----------------------
                # (qkv_phase for s+1 is emitted AFTER this loop: its
                # PSUM->SBUF copies on ACT would otherwise queue ahead of
                # LN2's Ln/Exp and stall the whole bn->rstd->stt chain)
                ot_all = bigp.tile([128, 8 * D], F32, name="ot_all", tag="ot_all")
                for hf in range(2):
                    st2h2 = smp.tile([128, 8], F32, name="st2h2", tag="st2h2")
                    t1s = []
                    for k in range(4):
                        tc_ = hf * 4 + k
                        ps = psM.tile([128, D], F32, name="mm", tag="mm")
                        for fc in range(4):
                            nc.tensor.matmul(
                                ps[:], lhsT=rTall[:, fc * L + tc_ * 128:fc * L + (tc_ + 1) * 128],
                                rhs=s_w2[:, fc * D:(fc + 1) * D],
                                start=(fc == 0), stop=False)
                        for dc in range(2):
                            nc.tensor.matmul(
                                ps[:], lhsT=z1nT(tc_, dc),
                                rhs=s_dg1[dc][:], start=False, stop=False)
                        nc.tensor.matmul(ps[:], lhsT=s_ones1[:], rhs=s_browz[:],
                                         start=False, stop=True)
                        st6 = smp.tile([128, 6], F32, name="st6", tag="st6")
                        nc.vector.bn_stats(st6[:], ps[:])
                        nc.vector.bn_aggr(st2h2[:, 2 * k:2 * k + 2], st6[:])
                        t1 = smp.tile([128, D], F32, name="t1", tag="t1", bufs=6)
                        nc.vector.scalar_tensor_tensor(
                            out=t1[:], in0=ps[:],
                            scalar=st2h2[:, 2 * k:2 * k + 1], in1=s_g2b[:],
                            op0=OP.subtract, op1=OP.mult)
                        t1s.append(t1)
                    lnv4b = smp.tile([128, 4], F32, name="lnv4b", tag="lnv4b")
                    nc.scalar.activation(out=lnv4b[:],
                                         in_=_bcast(st2h2[0:128, 0:1], 1, [[2, 4]]),
                                         func=AF.Ln, bias=s_eps[:, 0:1])
                    rstd4b = smp.tile([128, 4], F32, name="rstd4b", tag="rstd4b")
                    nc.scalar.activation(out=rstd4b[:], in_=lnv4b[:],
                                         func=AF.Exp, scale=-0.5)
                    for k in range(4):
                        tc_ = hf * 4 + k
                        nc.vector.scalar_tensor_tensor(
                            out=ot_all[:, tc_ * D:(tc_ + 1) * D], in0=t1s[k][:],
                            scalar=rstd4b[:, k:k + 1], in1=s_b2gb[:],
                            op0=OP.mult, op1=OP.add)
                # store on the scalar queue: its 1024 descriptors would
                # otherwise sit ahead of the next sample's mid-attention
                # OTh0 transpose on the sync queue
                nc.scalar.dma_start(
                    out=_mkap(out[s, 0:1, 0:1], 0,
                              [[D, 128], [128 * D, 8], [1, D]]),
                    in_=ot_all[:])

                if s + 1 < SPC:
                    xpose_x(s + 1)
                    qkv_phase(s + 1)

    _split_sync_waits(nc)
    _CACHE["nc"] = nc
    return nc


def _in_maps(X, Wq, Wk, Wv, Wo, ln1_g, ln1_b, W1, b1, W2, b2, ln2_g, ln2_b):
    X = np.asarray(X, dtype=np.float32)
    f32 = lambda a: np.asarray(a, dtype=np.float32)
    Wq, Wk, Wv, Wo = f32(Wq), f32(Wk), f32(Wv), f32(Wo)
    W1, W2 = f32(W1), f32(W2)
    ln1_g, ln1_b, b1, b2 = f32(ln1_g), f32(ln1_b), f32(b1), f32(b2)
    ln2_g, ln2_b = f32(ln2_g), f32(ln2_b)

    bf = ml_dtypes.bfloat16
    w1eff = (ln1_g[:, None] * W1)
    b1eff = (b1 + ln1_b @ W1)
    dg1 = np.zeros((2, 128, D), np.float32)
    for c in range(2):
        for i in range(128):
            dg1[c, i, c * 128 + i] = ln1_g[c * 128 + i]
    # masks, transposed (S.T orientation: keys on partitions, queries free)
    m2d = _mask2d()
    mt_core = np.stack([m2d[0:128, 64:384], m2d[128:256, 192:512]])
    m01v = (mt_core == 0.0).astype(np.float32)  # [2, 128q, 320k] 0/1
    # chunk 0 is the 64-left-extended 128-aligned chunk: its first 64 keys
    # are outside the reference window (always masked -> rows stay 0)
    mtail = np.zeros((2, 128, 384), np.float32)
    for qc in range(2):
        T = m01v[qc].T  # [320 k, 128 q]
        mtail[qc, 64:128, 0:128] = T[0:64]
        mtail[qc, 0:128, 128:256] = T[64:192]
        mtail[qc, 0:128, 256:384] = T[192:320]
    # b0 causal chunk mask: [k, q] upper-tri (k <= q within a 128 chunk)
    mtb0 = np.triu(np.ones((128, 128), np.float32))
    statics = {
        "wq": (Wq * (DH ** -0.5)).astype(bf),
        "wk": Wk.astype(bf),
        "wv": Wv.astype(bf),
        "wo": Wo.astype(bf),
        "w1": w1eff.astype(bf),
        "w2": W2.astype(bf),
        "dg1": dg1.astype(bf),
        "b1col": b1eff.reshape(4, 128).T.astype(np.float32).copy(),
        "browz": (ln1_b + b2)[None, :].astype(bf),
        "g2b": np.tile(ln2_g[None, :], (128, 1)).astype(np.float32),
        "b2gb": np.tile(ln2_b[None, :], (128, 1)).astype(np.float32),
        "ones1": np.ones((1, 128)).astype(bf),
        "mtail": mtail.astype(bf),
        "mtb0": mtb0.astype(bf),
    }

    in_maps = []
    for i in range(N_CORES):
        m = {"X": X[i * SPC:(i + 1) * SPC]}
        m.update(statics)
        in_maps.append(m)
    return in_maps


def kernel(**inputs):
    from concourse.bass_utils import run_bass_kernel_spmd
    nc = _build()
    res = run_bass_kernel_spmd(nc, _in_maps(**inputs), list(range(N_CORES)))
    return np.concatenate([res.results[i]["OUT"] for i in range(N_CORES)], axis=0)


def kernel_profiled(tmpdir=None, **inputs):
    from concourse.bass_utils import run_bass_kernel_spmd
    nc = _build()
    res = run_bass_kernel_spmd(nc, _in_maps(**inputs), list(range(N_CORES)),
                               trace=True, tmpdir=tmpdir)
    out = np.concatenate([res.results[i]["OUT"] for i in range(N_CORES)], axis=0)
    return out, res
